# revision 13
# baseline (speedup 1.0000x reference)
"""Sparse BertSelfAttention TRN2 kernel (8 NeuronCores, SPMD).

Sharding: core c -> (batch b = c//2, head-half = c%2).  Each core computes the
full attention for 6 of the 12 heads of one batch: output channels
[half*384, half*384+384) of out[b].  Host slices weights / builds index
tensors; device does gathers, projections, attention, scatters.

Math per core (O = 384 channel slice, heads h0..h0+5):
  xq = hidden[q_idx], xkv = hidden[kv_idx]                (indirect DMA gather)
  xqT, xkvT = transposes (PE identity transpose)
  qgT = (WqT_slice).T @ xqT + bq   [384, 1024]   (fp32r matmuls)
  kgT likewise; vg = xkvT.T @ WvT_slice + bv  [1024, 384] (+ ones cols -> 390)
  per head: S^T[j,i] = kg_h @ qg_h^T ; expS = exp(S^T/8 + mask_j)
            pv[0:64] = vg_h.T @ expS (ctx^T unnorm), pv[64] = rowsum
            ctx[i, d] = transpose(pv)[i, d] / rowsum[i]
  vmean_w = sum_j e^{mask_j} vg_j / sum_j e^{mask_j}  (ones/expmask matmul)
  out rows at q_idx <- ctx ; rows not in q_idx <- vmean_w  (indirect scatter)
"""
import threading

import numpy as np

B, T, H = 4, 2048, 768
NH, DH = 12, 64
KQ, KKV = 1024, 1024
O = 384          # output channels per core
NHC = 6          # heads per core
N_CORES = 8
N_SWDGE_Q = 4

_lock = threading.Lock()
_state = {}


def _indirect_dma(nc, out, out_offset, in_, in_offset, queue_idx):
    """nc.gpsimd.indirect_dma_start with a selectable SWDGE queue."""
    from concourse import mybir

    g = nc.gpsimd
    offset_ap_with_axis = in_offset or out_offset
    offset_ap = offset_ap_with_axis.ap
    offset_axis = offset_ap_with_axis.axis
    if in_offset:
        src_ap, dest_ap = in_, out
    else:
        src_ap, dest_ap = out, in_
    assert isinstance(src_ap.offset, int) and src_ap.offset == 0
    out_ap = g.lower_ap_dma(out, for_indirect_dma=True)
    in_ap = g.lower_ap_dma(in_, for_indirect_dma=True)
    assert len(in_ap) == 1 and len(out_ap) == 1
    offset_ap_l = g.lower_ap_dma(offset_ap)[0]
    in_ap.append(offset_ap_l)
    ap_shape = src_ap.shape
    coef = 1
    for i in range(offset_axis + 1, len(ap_shape)):
        coef *= ap_shape[i]
    dynamic_ap_info = mybir.DynamicAccessPatternInfo(
        c=0,
        actual_ap=dest_ap.ap,
        indirect_dim_max_index=ap_shape[offset_axis],
        offset_expr=[
            mybir.DynamicAccessPatternOffsetExpr(
                coef=coef,
                aff_expr=mybir.DynamicAccessPatternOffsetExprAffExpr(
                    kind="IndirectArgId", arg_id=1,
                ),
            )
        ],
    )
    if in_offset:
        in_ap[0].dynamic_ap_info = dynamic_ap_info
    else:
        out_ap[0].dynamic_ap_info = dynamic_ap_info
    qname = f"qPoolDynamic{queue_idx or ''}"
    return g.add_instruction(
        mybir.InstDMACopy(
            name=nc.get_next_instruction_name(),
            queue=qname,
            mode="Copy",
            ins=in_ap,
            outs=out_ap,
            oob_is_err=True,
            cce_op=mybir.AluOpType.bypass,
        )
    )


def _build(repeat=1, plain_gather=False, plain_scatter=False, phase=99):
    import concourse.bass as bass
    import concourse.bacc as bacc
    import concourse.tile as tile
    from concourse import mybir
    from concourse.masks import make_identity

    P = 128
    f32 = mybir.dt.float32
    f32r = mybir.dt.float32r
    i32 = mybir.dt.int32
    EXP = mybir.ActivationFunctionType.Exp

    nc = bacc.Bacc(None, target_bir_lowering=False, debug=False,
                   num_swdge_queues=N_SWDGE_Q)

    hidden = nc.dram_tensor("hidden", [T, H], f32, kind="ExternalInput")
    wqt = nc.dram_tensor("wqt", [H, O], f32, kind="ExternalInput")
    wkt = nc.dram_tensor("wkt", [H, O], f32, kind="ExternalInput")
    wvt = nc.dram_tensor("wvt", [H, O], f32, kind="ExternalInput")
    bq = nc.dram_tensor("bq", [O], f32, kind="ExternalInput")
    bk = nc.dram_tensor("bk", [O], f32, kind="ExternalInput")
    bv = nc.dram_tensor("bv", [O], f32, kind="ExternalInput")
    qidx = nc.dram_tensor("qidx", [KQ], i32, kind="ExternalInput")
    kvidx = nc.dram_tensor("kvidx", [KKV], i32, kind="ExternalInput")
    nqidx = nc.dram_tensor("nqidx", [T - KQ], i32, kind="ExternalInput")
    maskkv = nc.dram_tensor("maskkv", [KKV], f32, kind="ExternalInput")
    out_d = nc.dram_tensor("out", [T, O], f32, kind="ExternalOutput")

    NQT = KQ // P          # 8 q-row tiles
    NJT = KKV // P         # 8 kv-row tiles
    NHB = H // P           # 6 hidden-dim tiles
    NMO = O // P           # 3 output-channel tiles
    NNI = KQ // 512        # 2 query column tiles

    dmaq = [0]

    def nextq():
        dmaq[0] = (dmaq[0] + 1) % N_SWDGE_Q
        return dmaq[0]

    with tile.TileContext(nc) as tc:
      for rep in range(repeat):
        sfx = f"_{rep}"
        with (
            tc.tile_pool(name="const" + sfx, bufs=1) as const,
            tc.tile_pool(name="perm" + sfx, bufs=1) as perm,
            tc.tile_pool(name="ps" + sfx, bufs=1, space="PSUM") as ps,
        ):
            # ---------- constants ----------
            ident = const.tile([P, P], f32, name="ident")
            make_identity(nc, ident[:])
            identr = const.tile([P, P], f32r, name="identr")
            nc.vector.tensor_copy(out=identr[:], in_=ident[:])

            ones_f = const.tile([1, P], f32, name="ones_f")
            nc.vector.memset(ones_f[:], 1.0)
            ones1r = const.tile([1, P], f32r, name="ones1r")
            nc.vector.tensor_copy(out=ones1r[:], in_=ones_f[:])

            ones6 = const.tile([P, NHC], f32, name="ones6")
            nc.vector.memset(ones6[:], 1.0)

            # index / small tensors: [128, ntiles] column layout
            qidx_sb = const.tile([P, NQT], i32, name="qidx_sb")
            nc.sync.dma_start(out=qidx_sb[:], in_=bass.AP(qidx, 0, [[1, P], [P, NQT]]))
            kvidx_sb = const.tile([P, NJT], i32, name="kvidx_sb")
            nc.sync.dma_start(out=kvidx_sb[:], in_=bass.AP(kvidx, 0, [[1, P], [P, NJT]]))
            nqidx_sb = const.tile([P, NQT], i32, name="nqidx_sb")
            nc.sync.dma_start(out=nqidx_sb[:], in_=bass.AP(nqidx, 0, [[1, P], [P, NQT]]))
            maskkv_sb = const.tile([P, NJT], f32, name="maskkv_sb")
            nc.sync.dma_start(out=maskkv_sb[:], in_=bass.AP(maskkv, 0, [[1, P], [P, NJT]]))
            expmask_sb = const.tile([P, NJT], f32r, name="expmask_sb")
            nc.scalar.activation(expmask_sb[:], maskkv_sb[:], EXP)

            bq_sb = const.tile([P, NMO], f32, name="bq_sb")
            nc.sync.dma_start(out=bq_sb[:], in_=bass.AP(bq, 0, [[1, P], [P, NMO]]))
            bk_sb = const.tile([P, NMO], f32, name="bk_sb")
            nc.sync.dma_start(out=bk_sb[:], in_=bass.AP(bk, 0, [[1, P], [P, NMO]]))
            bv_sb = const.tile([1, O], f32r, name="bv_sb")
            nc.sync.dma_start(out=bv_sb[:], in_=bass.AP(bv, 0, [[O, 1], [1, O]]).bitcast(f32r))
            # bv broadcast to all partitions via ones-matmul
            pbv = ps.tile([P, O], f32, tag="tp", bufs=2, name="pbv")
            nc.tensor.matmul(pbv[:], ones1r[:], bv_sb[:], start=True, stop=True)
            bvb_sb = const.tile([P, O], f32, name="bvb_sb")
            nc.vector.tensor_copy(out=bvb_sb[:], in_=pbv[:])

            # ---------- persistent activation storage ----------
            qgT = perm.tile([P, NMO * KQ], f32r, name="qgT")
            kgT = perm.tile([P, NMO * KKV], f32r, name="kgT")
            vga = perm.tile([P, NJT * NHC * 65], f32r, name="vga")
            ctx_all = perm.tile([P, NQT * O], f32, name="ctx_all")

            with (
                tc.tile_pool(name="xph" + sfx, bufs=1) as xph,
                tc.tile_pool(name="gp" + sfx, bufs=3) as gp,
            ):
                # ---------- gather + transpose ----------
                xqT = xph.tile([P, NHB * KQ], f32r, name="xqT")
                xkvT = xph.tile([P, NHB * KKV], f32r, name="xkvT")
                for side, (idx_sb, n_t, xT) in enumerate(
                    ((qidx_sb, NQT, xqT), (kvidx_sb, NJT, xkvT))
                ):
                    for t in range(n_t):
                        xg = gp.tile([P, H], f32r, tag="xg", name=f"xg{side}_{t}")
                        if plain_gather:
                            nc.sync.dma_start(out=xg[:], in_=hidden[t * P:(t + 1) * P, :].bitcast(f32r))
                        else:
                            _indirect_dma(
                                nc, xg[:], None, hidden[:].bitcast(f32r),
                                bass.IndirectOffsetOnAxis(ap=idx_sb[:, t:t + 1], axis=0),
                                nextq(),
                            )
                        ptr = ps.tile([P, H], f32, tag="s", bufs=2, name=f"ptr{side}_{t}")
                        for hb in range(NHB):
                            nc.tensor.transpose(
                                ptr[:, hb * P:(hb + 1) * P].bitcast(f32r),
                                xg[:, hb * P:(hb + 1) * P], identr[:],
                            )
                        # one strided copy: psum [128, 6*128] -> xT cols {hb*1024 + t*128}
                        dst = bass.AP(xT.tensor, xT[:].offset + t * P,
                                      [xT[:].ap[0], [KQ, NHB], [1, P]])
                        src = bass.AP(ptr.tensor, ptr[:].offset,
                                      [ptr[:].ap[0], [P, NHB], [1, P]])
                        nc.vector.tensor_copy(out=dst, in_=src)

                # ---------- projections ----------
                if phase <= 1:
                    nc.sync.dma_start(out=out_d[0:P, :], in_=xqT[:, 0:O].bitcast(f32))
                    continue
                with tc.tile_pool(name="wp" + sfx, bufs=1) as wp:
                    wq_sb = wp.tile([P, NHB * O], f32r, name="wq_sb")
                    wk_sb = wp.tile([P, NHB * O], f32r, name="wk_sb")
                    wv_sb = wp.tile([P, NHB * O], f32r, name="wv_sb")
                    for kh in range(NHB):
                        nc.sync.dma_start(out=wq_sb[:, kh * O:(kh + 1) * O],
                                          in_=wqt[kh * P:(kh + 1) * P, :].bitcast(f32r))
                        nc.sync.dma_start(out=wk_sb[:, kh * O:(kh + 1) * O],
                                          in_=wkt[kh * P:(kh + 1) * P, :].bitcast(f32r))
                        nc.sync.dma_start(out=wv_sb[:, kh * O:(kh + 1) * O],
                                          in_=wvt[kh * P:(kh + 1) * P, :].bitcast(f32r))

                    # qgT / kgT: [o, i] layout
                    for w_sb, b_sb, gT, xT in ((wq_sb, bq_sb, qgT, xqT),
                                               (wk_sb, bk_sb, kgT, xkvT)):
                        for mo in range(NMO):
                            for ni in range(NNI):
                                pp = ps.tile([P, 512], f32, tag="s", bufs=2,
                                             name=f"pp{id(w_sb) % 97}_{mo}_{ni}")
                                for kh in range(NHB):
                                    nc.tensor.matmul(
                                        pp[:],
                                        w_sb[:, kh * O + mo * P: kh * O + (mo + 1) * P],
                                        xT[:, kh * KQ + ni * 512: kh * KQ + (ni + 1) * 512],
                                        start=(kh == 0), stop=(kh == NHB - 1),
                                    )
                                nc.vector.tensor_scalar_add(
                                    gT[:, mo * KQ + ni * 512: mo * KQ + (ni + 1) * 512],
                                    pp[:], b_sb[:, mo:mo + 1],
                                )

                    # vg: [j, o] layout + ones columns (65-stride per head)
                    for mj in range(NJT):
                        pv_ = ps.tile([P, O], f32, tag="s", bufs=2, name=f"pvv{mj}")
                        for kh in range(NHB):
                            nc.tensor.matmul(
                                pv_[:],
                                xkvT[:, kh * KKV + mj * P: kh * KKV + (mj + 1) * P],
                                wv_sb[:, kh * O:(kh + 1) * O],
                                start=(kh == 0), stop=(kh == NHB - 1),
                            )
                        base = mj * NHC * 65
                        nc.vector.tensor_copy(
                            out=bass.AP(vga.tensor, vga[:].offset + base + 64,
                                        [vga[:].ap[0], [65, NHC], [1, 1]]),
                            in_=bass.AP(ones6.tensor, ones6[:].offset,
                                        [ones6[:].ap[0], [1, NHC], [1, 1]]),
                        )
                        nc.vector.tensor_tensor(
                            out=bass.AP(vga.tensor, vga[:].offset + base,
                                        [vga[:].ap[0], [65, NHC], [1, DH]]),
                            in0=bass.AP(pv_.tensor, pv_[:].offset,
                                        [pv_[:].ap[0], [DH, NHC], [1, DH]]),
                            in1=bass.AP(bvb_sb.tensor, bvb_sb[:].offset,
                                        [bvb_sb[:].ap[0], [DH, NHC], [1, DH]]),
                            op=mybir.AluOpType.add,
                        )

            if phase <= 2:
                nc.sync.dma_start(out=out_d[0:P, :], in_=qgT[:, 0:O].bitcast(f32))
                continue
            # ---------- attention ----------
            with tc.tile_pool(name="ap" + sfx, bufs=1) as apool, \
                 tc.tile_pool(name="ep" + sfx, bufs=3) as ep, \
                 tc.tile_pool(name="cp" + sfx, bufs=3) as cp, \
                 tc.tile_pool(name="pvp" + sfx, bufs=2, space="PSUM") as pvp:
                for h in range(NHC):
                    r, sub = h // 2, h % 2
                    o0 = sub * DH
                    pv_ps = [
                        pvp.tile([65, 512], f32, tag="pv", name=f"pvps{h}_{ni}")
                        for ni in range(NNI)
                    ]
                    for mj in range(NJT):
                        s_ps = ps.tile([P, KQ], f32, tag="s", bufs=2, name=f"sps{h}_{mj}")
                        for ni in range(NNI):
                            nc.tensor.matmul(
                                s_ps[:, ni * 512:(ni + 1) * 512],
                                kgT[o0:o0 + DH, r * KKV + mj * P: r * KKV + (mj + 1) * P],
                                qgT[o0:o0 + DH, r * KQ + ni * 512: r * KQ + (ni + 1) * 512],
                                start=True, stop=True,
                            )
                        expS = ep.tile([P, KQ], f32r, tag="expS", name=f"expS{h}_{mj}")
                        nc.scalar.activation(expS[:], s_ps[:], EXP,
                                             bias=maskkv_sb[:, mj:mj + 1], scale=0.125)
                        for ni in range(NNI):
                            nc.tensor.matmul(
                                pv_ps[ni][:],
                                vga[:, (mj * NHC + h) * 65:(mj * NHC + h) * 65 + 65],
                                expS[:, ni * 512:(ni + 1) * 512],
                                start=(mj == 0), stop=(mj == NJT - 1),
                            )
                    for ni in range(NNI):
                        # [96, 512] so transposed blocks are 32-multiples;
                        # rows 65:96 are never written (garbage, never read).
                        ctxT = cp.tile([96, 512], f32r, tag="ctxT", name=f"ctxT{h}_{ni}")
                        nc.vector.tensor_copy(out=ctxT[0:65, :], in_=pv_ps[ni][:])
                        pt4 = ps.tile([P, 4 * 96], f32, tag="tp", bufs=2,
                                      name=f"pt4{h}_{ni}")
                        for blk in range(4):
                            nc.tensor.transpose(
                                pt4[:, blk * 96:(blk + 1) * 96].bitcast(f32r),
                                ctxT[:, blk * P:(blk + 1) * P],
                                identr[:96, :96],
                            )
                        rec4 = cp.tile([P, 4], f32, tag="rec4", name=f"rec4{h}_{ni}")
                        nc.vector.reciprocal(
                            rec4[:],
                            bass.AP(pt4.tensor, pt4[:].offset + DH,
                                    [pt4[:].ap[0], [96, 4], [1, 1]]),
                        )
                        nc.vector.tensor_tensor(
                            out=bass.AP(ctx_all.tensor,
                                        ctx_all[:].offset + (ni * 4) * O + h * DH,
                                        [ctx_all[:].ap[0], [O, 4], [1, DH]]),
                            in0=bass.AP(pt4.tensor, pt4[:].offset,
                                        [pt4[:].ap[0], [96, 4], [1, DH]]),
                            in1=bass.AP(rec4.tensor, rec4[:].offset,
                                        [rec4[:].ap[0], [1, 4], [0, DH]]),
                            op=mybir.AluOpType.mult,
                        )

                if phase <= 3:
                    nc.sync.dma_start(out=out_d[0:P, :], in_=ctx_all[:, 0:O])
                    continue
                # ---------- weighted mean of v for non-query rows ----------
                pm = ps.tile([1, NHC * 65], f32, tag="tp", bufs=2, name="pm")
                for mj in range(NJT):
                    nc.tensor.matmul(
                        pm[:], expmask_sb[:, mj:mj + 1],
                        vga[:, mj * NHC * 65:(mj + 1) * NHC * 65],
                        start=(mj == 0), stop=(mj == NJT - 1),
                    )
                vsum = cp.tile([1, NHC * 65], f32r, tag="vsum", name="vsum")
                nc.vector.tensor_copy(out=vsum[:], in_=pm[:])
                rec1 = cp.tile([1, 1], f32, tag="rec1", name="rec1")
                nc.vector.reciprocal(rec1[:], vsum[:1, 64:65])
                vmean = cp.tile([1, O], f32r, tag="vmean", name="vmean")
                nc.vector.tensor_scalar_mul(
                    bass.AP(vmean.tensor, vmean[:].offset,
                            [vmean[:].ap[0], [DH, NHC], [1, DH]]),
                    bass.AP(vsum.tensor, vsum[:].offset,
                            [vsum[:].ap[0], [65, NHC], [1, DH]]),
                    rec1[:, :1],
                )
                pmb = ps.tile([P, O], f32, tag="tp", bufs=2, name="pmb")
                nc.tensor.matmul(pmb[:], ones1r[:], vmean[:], start=True, stop=True)
                vmb = cp.tile([P, O], f32, tag="vmb", name="vmb")
                nc.vector.tensor_copy(out=vmb[:], in_=pmb[:])

                # ---------- scatters ----------
                for t in range(NQT):
                    if plain_scatter:
                        nc.sync.dma_start(out=out_d[t * P:(t + 1) * P, :], in_=vmb[:].to_broadcast([P, O]))
                        nc.sync.dma_start(out=out_d[(8 + t) * P:(9 + t) * P, :], in_=ctx_all[:, t * O:(t + 1) * O])
                    else:
                        _indirect_dma(
                            nc, out_d[:],
                            bass.IndirectOffsetOnAxis(ap=nqidx_sb[:, t:t + 1], axis=0),
                            vmb[:], None, nextq(),
                        )
                        _indirect_dma(
                            nc, out_d[:],
                            bass.IndirectOffsetOnAxis(ap=qidx_sb[:, t:t + 1], axis=0),
                            ctx_all[:, t * O:(t + 1) * O], None, nextq(),
                        )

    nc.compile()
    return nc


def _get_runner():
    """Build (once) a reusable jitted SPMD callable over 8 cores."""
    with _lock:
        if "runner" in _state:
            return _state["runner"]

        import jax
        from jax.sharding import Mesh, PartitionSpec
        from jax.experimental.shard_map import shard_map
        from concourse import mybir
        from concourse import bass2jax

        nc = _build()
        bass2jax.install_neuronx_cc_hook()

        partition_name = (
            nc.partition_id_tensor.name if nc.partition_id_tensor else None
        )
        in_names, out_names, out_avals, zero_outs = [], [], [], []
        for alloc in nc.m.functions[0].allocations:
            if not isinstance(alloc, mybir.MemoryLocationSet):
                continue
            name = alloc.memorylocations[0].name
            if alloc.kind == "ExternalInput":
                if name != partition_name:
                    in_names.append(name)
            elif alloc.kind == "ExternalOutput":
                out_names.append(name)
                shape = tuple(alloc.tensor_shape)
                dtype = mybir.dt.np(alloc.dtype)
                out_avals.append(jax.core.ShapedArray(shape, dtype))
                zero_outs.append(np.zeros(shape, dtype))
        n_params = len(in_names)
        all_names = in_names + out_names
        if partition_name is not None:
            all_names = all_names + [partition_name]

        def _body(*args):
            operands = list(args)
            if partition_name is not None:
                operands.append(bass2jax.partition_id_tensor())
            outs = bass2jax._bass_exec_p.bind(
                *operands,
                out_avals=tuple(out_avals),
                in_names=tuple(all_names),
                out_names=tuple(out_names),
                lowering_input_output_aliases=(),
                sim_require_finite=True,
                sim_require_nnan=True,
                nc=nc,
            )
            return tuple(outs)

        try:
            devices = jax.devices("axon")[:N_CORES]
        except RuntimeError:
            devices = jax.devices()[:N_CORES]
        mesh = Mesh(np.asarray(devices), ("core",))
        n_out = len(out_names)
        sharded = jax.jit(
            shard_map(
                _body, mesh=mesh,
                in_specs=(PartitionSpec("core"),) * (n_params + n_out),
                out_specs=(PartitionSpec("core"),) * n_out,
                check_rep=False,
            ),
            donate_argnums=tuple(range(n_params, n_params + n_out)),
            keep_unused=True,
        )

        def run(in_maps):
            concat_in = [
                np.concatenate([np.asarray(in_maps[c][nm]) for c in range(N_CORES)],
                               axis=0)
                for nm in in_names
            ]
            concat_zero = [
                np.concatenate([z for _ in range(N_CORES)], axis=0) for z in zero_outs
            ]
            out_arrs = sharded(*concat_in, *concat_zero)
            out_arrs = [np.asarray(a) for a in out_arrs]
            results = []
            for c in range(N_CORES):
                m = {}
                for i, nm in enumerate(out_names):
                    sh0 = out_avals[i].shape[0]
                    m[nm] = out_arrs[i][c * sh0:(c + 1) * sh0]
                results.append(m)
            return results

        _state["runner"] = run
        return run


def _shard_inputs(hidden_states, attention_mask, Wq, bq, Wk, bk, Wv, bv,
                  q_indices, kv_indices):
    in_maps = []
    all_tok = np.arange(T, dtype=np.int32)
    for c in range(N_CORES):
        b, half = c // 2, c % 2
        o0 = half * O
        qi = np.ascontiguousarray(q_indices[b].astype(np.int32))
        kvi = np.ascontiguousarray(kv_indices[b].astype(np.int32))
        nqi = np.setdiff1d(all_tok, qi).astype(np.int32)
        in_maps.append({
            "hidden": np.ascontiguousarray(hidden_states[b], dtype=np.float32),
            "wqt": np.ascontiguousarray(Wq[o0:o0 + O, :].T, dtype=np.float32),
            "wkt": np.ascontiguousarray(Wk[o0:o0 + O, :].T, dtype=np.float32),
            "wvt": np.ascontiguousarray(Wv[o0:o0 + O, :].T, dtype=np.float32),
            "bq": np.ascontiguousarray(bq[o0:o0 + O], dtype=np.float32),
            "bk": np.ascontiguousarray(bk[o0:o0 + O], dtype=np.float32),
            "bv": np.ascontiguousarray(bv[o0:o0 + O], dtype=np.float32),
            "qidx": qi,
            "kvidx": kvi,
            "nqidx": nqi,
            "maskkv": np.ascontiguousarray(
                np.asarray(attention_mask, dtype=np.float32)[b, 0, 0, kvi]),
        })
    return in_maps


def kernel(hidden_states, attention_mask, Wq, bq, Wk, bk, Wv, bv,
           q_indices, kv_indices):
    run = _get_runner()
    in_maps = _shard_inputs(hidden_states, attention_mask, Wq, bq, Wk, bk, Wv, bv,
                            q_indices, kv_indices)
    results = run(in_maps)
    out = np.empty((B, T, NH * DH), dtype=np.float32)
    for c in range(N_CORES):
        b, half = c // 2, c % 2
        out[b, :, half * O:(half + 1) * O] = results[c]["out"]
    return out


# revision 23
# speedup vs baseline: 436.9729x; 436.9729x over previous
"""Sparse BertSelfAttention TRN2 kernel (8 NeuronCores, SPMD).

Sharding: core c -> (batch b = c//2, head-half = c%2).  Each core computes the
full attention for 6 of the 12 heads of one batch: output channels
[half*384, half*384+384) of out[b].  Host slices weights / builds index
tensors; device does gathers, projections, attention, scatters.

Math per core (O = 384 channel slice, heads h0..h0+5):
  xq = hidden[q_idx], xkv = hidden[kv_idx]                (indirect DMA gather)
  xqT, xkvT = transposes (PE identity transpose)
  qgT = (WqT_slice).T @ xqT + bq   [384, 1024]   (fp32r matmuls)
  kgT likewise; vg = xkvT.T @ WvT_slice + bv  [1024, 384] (+ ones cols -> 390)
  per head: S^T[j,i] = kg_h @ qg_h^T ; expS = exp(S^T/8 + mask_j)
            pv[0:64] = vg_h.T @ expS (ctx^T unnorm), pv[64] = rowsum
            ctx[i, d] = transpose(pv)[i, d] / rowsum[i]
  vmean_w = sum_j e^{mask_j} vg_j / sum_j e^{mask_j}  (ones/expmask matmul)
  out rows at q_idx <- ctx ; rows not in q_idx <- vmean_w  (indirect scatter)
"""
import threading

import numpy as np

B, T, H = 4, 2048, 768
NH, DH = 12, 64
KQ, KKV = 1024, 1024
O = 384          # output channels per core
NHC = 6          # heads per core
N_CORES = 8
N_SWDGE_Q = 4

_lock = threading.Lock()
_state = {}


def _indirect_dma(nc, out, out_offset, in_, in_offset, queue_idx,
                  shape_override=None):
    """nc.gpsimd.indirect_dma_start with a selectable SWDGE queue."""
    from concourse import mybir

    g = nc.gpsimd
    offset_ap_with_axis = in_offset or out_offset
    offset_ap = offset_ap_with_axis.ap
    offset_axis = offset_ap_with_axis.axis
    if in_offset:
        src_ap, dest_ap = in_, out
    else:
        src_ap, dest_ap = out, in_
    assert isinstance(src_ap.offset, int) and src_ap.offset == 0
    out_ap = g.lower_ap_dma(out, for_indirect_dma=True)
    in_ap = g.lower_ap_dma(in_, for_indirect_dma=True)
    assert len(in_ap) == 1 and len(out_ap) == 1
    offset_ap_l = g.lower_ap_dma(offset_ap)[0]
    in_ap.append(offset_ap_l)
    ap_shape = shape_override if shape_override is not None else src_ap.shape
    coef = 1
    for i in range(offset_axis + 1, len(ap_shape)):
        coef *= ap_shape[i]
    dynamic_ap_info = mybir.DynamicAccessPatternInfo(
        c=0,
        actual_ap=dest_ap.ap,
        indirect_dim_max_index=ap_shape[offset_axis],
        offset_expr=[
            mybir.DynamicAccessPatternOffsetExpr(
                coef=coef,
                aff_expr=mybir.DynamicAccessPatternOffsetExprAffExpr(
                    kind="IndirectArgId", arg_id=1,
                ),
            )
        ],
    )
    if in_offset:
        in_ap[0].dynamic_ap_info = dynamic_ap_info
    else:
        out_ap[0].dynamic_ap_info = dynamic_ap_info
    qname = f"qPoolDynamic{queue_idx or ''}"
    return g.add_instruction(
        mybir.InstDMACopy(
            name=nc.get_next_instruction_name(),
            queue=qname,
            mode="Copy",
            ins=in_ap,
            outs=out_ap,
            oob_is_err=True,
            cce_op=mybir.AluOpType.bypass,
        )
    )


def _build(repeat=1, plain_gather=False, plain_scatter=False, phase=99, scat=3):
    import concourse.bass as bass
    import concourse.bacc as bacc
    import concourse.tile as tile
    from concourse import mybir
    from concourse.masks import make_identity

    P = 128
    f32 = mybir.dt.float32
    f32r = mybir.dt.float32r
    i32 = mybir.dt.int32
    EXP = mybir.ActivationFunctionType.Exp

    nc = bacc.Bacc(None, target_bir_lowering=False, debug=False,
                   num_swdge_queues=N_SWDGE_Q)

    hidden = nc.dram_tensor("hidden", [T, H], f32, kind="ExternalInput")
    wqt = nc.dram_tensor("wqt", [H, O], f32, kind="ExternalInput")
    wkt = nc.dram_tensor("wkt", [H, O], f32, kind="ExternalInput")
    wvt = nc.dram_tensor("wvt", [H, O], f32, kind="ExternalInput")
    bq = nc.dram_tensor("bq", [O], f32, kind="ExternalInput")
    bk = nc.dram_tensor("bk", [O], f32, kind="ExternalInput")
    bv = nc.dram_tensor("bv", [O], f32, kind="ExternalInput")
    qidx = nc.dram_tensor("qidx", [KQ], i32, kind="ExternalInput")
    kvidx = nc.dram_tensor("kvidx", [KKV], i32, kind="ExternalInput")
    nqidx = nc.dram_tensor("nqidx", [T - KQ], i32, kind="ExternalInput")
    maskkv = nc.dram_tensor("maskkv", [KKV], f32, kind="ExternalInput")
    out_d = nc.dram_tensor("out", [T, O], f32, kind="ExternalOutput")
    outs_dbg = None

    NQT = KQ // P          # 8 q-row tiles
    NJT = KKV // P         # 8 kv-row tiles
    NHB = H // P           # 6 hidden-dim tiles
    NMO = O // P           # 3 output-channel tiles
    NNI = KQ // 512        # 2 query column tiles

    dmaq = [0]

    def nextq():
        dmaq[0] = (dmaq[0] + 1) % N_SWDGE_Q
        return dmaq[0]

    with tile.TileContext(nc) as tc:
      for rep in range(repeat):
        sfx = f"_{rep}"
        with (
            tc.tile_pool(name="const" + sfx, bufs=1) as const,
            tc.tile_pool(name="perm" + sfx, bufs=1) as perm,
            tc.tile_pool(name="ps" + sfx, bufs=1, space="PSUM") as ps,
        ):
            # ---------- constants ----------
            ident = const.tile([P, P], f32, name="ident")
            make_identity(nc, ident[:])
            identr = const.tile([P, P], f32r, name="identr")
            nc.vector.tensor_copy(out=identr[:], in_=ident[:])

            ones_f = const.tile([1, P], f32, name="ones_f")
            nc.vector.memset(ones_f[:], 1.0)
            ones1r = const.tile([1, P], f32r, name="ones1r")
            nc.vector.tensor_copy(out=ones1r[:], in_=ones_f[:])

            ones6 = const.tile([P, NHC], f32, name="ones6")
            nc.vector.memset(ones6[:], 1.0)

            # index / small tensors: [128, ntiles] column layout
            qidx_sb = const.tile([P, NQT], i32, name="qidx_sb")
            nc.sync.dma_start(out=qidx_sb[:], in_=bass.AP(qidx, 0, [[1, P], [P, NQT]]))
            kvidx_sb = const.tile([P, NJT], i32, name="kvidx_sb")
            nc.sync.dma_start(out=kvidx_sb[:], in_=bass.AP(kvidx, 0, [[1, P], [P, NJT]]))
            nqidx_sb = const.tile([P, NQT], i32, name="nqidx_sb")
            nc.sync.dma_start(out=nqidx_sb[:], in_=bass.AP(nqidx, 0, [[1, P], [P, NQT]]))
            maskkv_sb = const.tile([P, NJT], f32, name="maskkv_sb")
            nc.sync.dma_start(out=maskkv_sb[:], in_=bass.AP(maskkv, 0, [[1, P], [P, NJT]]))
            expmask_sb = const.tile([P, NJT], f32r, name="expmask_sb")
            nc.scalar.activation(expmask_sb[:], maskkv_sb[:], EXP)

            bq_sb = const.tile([P, NMO], f32, name="bq_sb")
            nc.sync.dma_start(out=bq_sb[:], in_=bass.AP(bq, 0, [[1, P], [P, NMO]]))
            bk_sb = const.tile([P, NMO], f32, name="bk_sb")
            nc.sync.dma_start(out=bk_sb[:], in_=bass.AP(bk, 0, [[1, P], [P, NMO]]))
            bv_sb = const.tile([1, O], f32r, name="bv_sb")
            nc.sync.dma_start(out=bv_sb[:], in_=bass.AP(bv, 0, [[O, 1], [1, O]]).bitcast(f32r))
            # bv broadcast to all partitions via ones-matmul
            pbv = ps.tile([P, O], f32, tag="pj", bufs=2, name="pbv")
            nc.tensor.matmul(pbv[:], ones1r[:], bv_sb[:], start=True, stop=True)
            bvb_sb = const.tile([P, O], f32, name="bvb_sb")
            nc.vector.tensor_copy(out=bvb_sb[:], in_=pbv[:])

            # ---------- persistent activation storage ----------
            qgT = perm.tile([P, NMO * KQ], f32r, name="qgT")
            kgT = perm.tile([P, NMO * KKV], f32r, name="kgT")
            vga = perm.tile([P, NJT * NHC * 65], f32r, name="vga")
            ctx_all = perm.tile([P, NQT * O], f32, name="ctx_all")

            with (
                tc.tile_pool(name="xph" + sfx, bufs=1) as xph,
                tc.tile_pool(name="gp" + sfx, bufs=4) as gp,
            ):
                # ---------- gather + transpose ----------
                xqT = xph.tile([P, NHB * KQ], f32r, name="xqT")
                xkvT = xph.tile([P, NHB * KKV], f32r, name="xkvT")
                with tc.tile_pool(name="wp" + sfx, bufs=1) as wp:
                    wq_sb = wp.tile([P, NHB * O], f32r, name="wq_sb")
                    wk_sb = wp.tile([P, NHB * O], f32r, name="wk_sb")
                    wv_sb = wp.tile([P, NHB * O], f32r, name="wv_sb")
                    for kh in range(NHB):
                        nc.sync.dma_start(out=wq_sb[:, kh * O:(kh + 1) * O],
                                          in_=wqt[kh * P:(kh + 1) * P, :].bitcast(f32r))
                        nc.sync.dma_start(out=wk_sb[:, kh * O:(kh + 1) * O],
                                          in_=wkt[kh * P:(kh + 1) * P, :].bitcast(f32r))
                        nc.sync.dma_start(out=wv_sb[:, kh * O:(kh + 1) * O],
                                          in_=wvt[kh * P:(kh + 1) * P, :].bitcast(f32r))

                    def emit_gather(side, t):
                        idx_sb, xT = ((qidx_sb, xqT), (kvidx_sb, xkvT))[side]
                        xg = gp.tile([P, H], f32r, tag="xg", name=f"xg{side}_{t}")
                        if plain_gather:
                            nc.sync.dma_start(out=xg[:],
                                              in_=hidden[t * P:(t + 1) * P, :].bitcast(f32r))
                        else:
                            _indirect_dma(
                                nc, xg[:], None, hidden[:].bitcast(f32r),
                                bass.IndirectOffsetOnAxis(ap=idx_sb[:, t:t + 1], axis=0),
                                nextq(),
                            )
                        ptr = ps.tile([P, H], f32, tag="s", bufs=2,
                                      name=f"ptr{side}_{t}")
                        for hb in range(NHB):
                            nc.tensor.transpose(
                                ptr[:, hb * P:(hb + 1) * P].bitcast(f32r),
                                xg[:, hb * P:(hb + 1) * P], identr[:],
                            )
                        nc.vector.tensor_copy(out=xT[:, t * H:(t + 1) * H], in_=ptr[:])

                    def emit_qk_proj(ni):
                        for mo in range(NMO):
                            for w_sb, b_sb, gT, xT in ((wq_sb, bq_sb, qgT, xqT),
                                                       (wk_sb, bk_sb, kgT, xkvT)):
                                pp = ps.tile([P, 512], f32, tag="pj", bufs=2,
                                             name=f"pp{id(w_sb) % 97}_{mo}_{ni}")
                                for kh in range(NHB):
                                    rhs = bass.AP(
                                        xT.tensor,
                                        xT[:].offset + (ni * 4) * H + kh * P,
                                        [xT[:].ap[0], [H, 4], [1, P]],
                                    )
                                    nc.tensor.matmul(
                                        pp[:],
                                        w_sb[:, kh * O + mo * P: kh * O + (mo + 1) * P],
                                        rhs,
                                        start=(kh == 0), stop=(kh == NHB - 1),
                                    )
                                nc.vector.tensor_scalar_add(
                                    gT[:, mo * KQ + ni * 512: mo * KQ + (ni + 1) * 512],
                                    pp[:], b_sb[:, mo:mo + 1],
                                )

                    def emit_v_proj(mj):
                        pv_ = ps.tile([P, O], f32, tag="pj", bufs=2, name=f"pvv{mj}")
                        for kh in range(NHB):
                            nc.tensor.matmul(
                                pv_[:],
                                xkvT[:, mj * H + kh * P: mj * H + (kh + 1) * P],
                                wv_sb[:, kh * O:(kh + 1) * O],
                                start=(kh == 0), stop=(kh == NHB - 1),
                            )
                        base = mj * NHC * 65
                        nc.vector.tensor_copy(
                            out=bass.AP(vga.tensor, vga[:].offset + base + 64,
                                        [vga[:].ap[0], [65, NHC], [1, 1]]),
                            in_=bass.AP(ones6.tensor, ones6[:].offset,
                                        [ones6[:].ap[0], [1, NHC], [1, 1]]),
                        )
                        nc.vector.tensor_tensor(
                            out=bass.AP(vga.tensor, vga[:].offset + base,
                                        [vga[:].ap[0], [65, NHC], [1, DH]]),
                            in0=bass.AP(pv_.tensor, pv_[:].offset,
                                        [pv_[:].ap[0], [DH, NHC], [1, DH]]),
                            in1=bass.AP(bvb_sb.tensor, bvb_sb[:].offset,
                                        [bvb_sb[:].ap[0], [DH, NHC], [1, DH]]),
                            op=mybir.AluOpType.add,
                        )

                    for t in range(NQT):
                        emit_gather(0, t)
                        emit_gather(1, t)
                        emit_v_proj(t)
                        if t == 3:
                            emit_qk_proj(0)
                        if t == 7:
                            emit_qk_proj(1)
                    if phase <= 1:
                        nc.sync.dma_start(out=out_d[0:P, :], in_=xqT[:, 0:O].bitcast(f32))
                        continue

            if phase <= 2:
                nc.sync.dma_start(out=out_d[0:P, :], in_=qgT[:, 0:O].bitcast(f32))
                continue
            # ---------- attention ----------
            with tc.tile_pool(name="ap" + sfx, bufs=1) as apool, \
                 tc.tile_pool(name="ep" + sfx, bufs=3) as ep, \
                 tc.tile_pool(name="cp" + sfx, bufs=3) as cp, \
                 tc.tile_pool(name="pvp" + sfx, bufs=2, space="PSUM") as pvp:
                # ---------- weighted mean of v -> fill all output rows ----------
                pm = ps.tile([1, NHC * 65], f32, tag="pj", bufs=2, name="pm")
                for mj in range(NJT):
                    nc.tensor.matmul(
                        pm[:], expmask_sb[:, mj:mj + 1],
                        vga[:, mj * NHC * 65:(mj + 1) * NHC * 65],
                        start=(mj == 0), stop=(mj == NJT - 1),
                    )
                vsum = cp.tile([1, NHC * 65], f32r, tag="vsum", name="vsum")
                nc.vector.tensor_copy(out=vsum[:], in_=pm[:])
                rec1 = cp.tile([1, 1], f32, tag="rec1", name="rec1")
                nc.vector.reciprocal(rec1[:], vsum[:1, 64:65])
                vmean = cp.tile([1, O], f32r, tag="vmean", name="vmean")
                nc.vector.tensor_scalar_mul(
                    bass.AP(vmean.tensor, vmean[:].offset,
                            [vmean[:].ap[0], [DH, NHC], [1, DH]]),
                    bass.AP(vsum.tensor, vsum[:].offset,
                            [vsum[:].ap[0], [65, NHC], [1, DH]]),
                    rec1[:, :1],
                )
                pmb = ps.tile([P, O], f32, tag="pj", bufs=2, name="pmb")
                nc.tensor.matmul(pmb[:], ones1r[:], vmean[:], start=True, stop=True)
                vmb = cp.tile([P, O], f32, tag="vmb", name="vmb")
                nc.vector.tensor_copy(out=vmb[:], in_=pmb[:])
                if scat != 0:
                    # dense fill: every output row <- vmean (ctx rows overwritten later)
                    nc.sync.dma_start(
                        out=bass.AP(out_d, 0, [[O, P], [O * P, T // P], [1, O]]),
                        in_=bass.AP(vmb.tensor, vmb[:].offset,
                                    [vmb[:].ap[0], [0, T // P], [1, O]]),
                    )

                for h in range(NHC):
                    r, sub = h // 2, h % 2
                    o0 = sub * DH
                    pv_ps = [
                        pvp.tile([65, 512], f32, tag="pv", name=f"pvps{h}_{ni}")
                        for ni in range(NNI)
                    ]
                    for mj in range(NJT):
                        s_ps = ps.tile([P, KQ], f32, tag="s", bufs=2, name=f"sps{h}_{mj}")
                        for ni in range(NNI):
                            nc.tensor.matmul(
                                s_ps[:, ni * 512:(ni + 1) * 512],
                                kgT[o0:o0 + DH, r * KKV + mj * P: r * KKV + (mj + 1) * P],
                                qgT[o0:o0 + DH, r * KQ + ni * 512: r * KQ + (ni + 1) * 512],
                                start=True, stop=True,
                            )
                        expS = ep.tile([P, KQ], f32r, tag="expS", name=f"expS{h}_{mj}")
                        nc.scalar.activation(expS[:], s_ps[:], EXP,
                                             bias=maskkv_sb[:, mj:mj + 1], scale=0.125)
                        for ni in range(NNI):
                            nc.tensor.matmul(
                                pv_ps[ni][:],
                                vga[:, (mj * NHC + h) * 65:(mj * NHC + h) * 65 + 65],
                                expS[:, ni * 512:(ni + 1) * 512],
                                start=(mj == 0), stop=(mj == NJT - 1),
                            )
                    for ni in range(NNI):
                        # [96, 512] so transposed blocks are 32-multiples;
                        # rows 65:96 are never written (garbage, never read).
                        ctxT = cp.tile([96, 512], f32r, tag="ctxT", name=f"ctxT{h}_{ni}")
                        nc.vector.tensor_copy(out=ctxT[0:65, :], in_=pv_ps[ni][:])
                        pt4 = ps.tile([P, 4 * 96], f32, tag="pj", bufs=2,
                                      name=f"pt4{h}_{ni}")
                        for blk in range(4):
                            nc.tensor.transpose(
                                pt4[:, blk * 96:(blk + 1) * 96].bitcast(f32r),
                                ctxT[:, blk * P:(blk + 1) * P],
                                identr[:96, :96],
                            )
                        rec4 = cp.tile([P, 4], f32, tag="rec4", name=f"rec4{h}_{ni}")
                        nc.vector.reciprocal(
                            rec4[:],
                            bass.AP(pt4.tensor, pt4[:].offset + DH,
                                    [pt4[:].ap[0], [96, 4], [1, 1]]),
                        )
                        nc.vector.tensor_tensor(
                            out=bass.AP(ctx_all.tensor,
                                        ctx_all[:].offset + (ni * 4) * O + h * DH,
                                        [ctx_all[:].ap[0], [O, 4], [1, DH]]),
                            in0=bass.AP(pt4.tensor, pt4[:].offset,
                                        [pt4[:].ap[0], [96, 4], [1, DH]]),
                            in1=bass.AP(rec4.tensor, rec4[:].offset,
                                        [rec4[:].ap[0], [1, 4], [0, DH]]),
                            op=mybir.AluOpType.mult,
                        )

                if phase <= 3:
                    nc.sync.dma_start(out=out_d[0:P, :], in_=ctx_all[:, 0:O])
                    continue
                # ---------- ctx scatters ----------
                for t in range(NQT):
                    if scat == 0:
                        continue
                    if plain_scatter:
                        nc.sync.dma_start(out=out_d[(8 + t) * P:(9 + t) * P, :],
                                          in_=ctx_all[:, t * O:(t + 1) * O])
                    else:
                        # static out AP covers only 128 rows; real rows come
                        # from the offset table (keeps dep region + cost sane)
                        _indirect_dma(
                            nc,
                            bass.AP(out_d, 0, [[O, P], [1, O]],
                                    dep_tracking_offset=t * P * O),
                            bass.IndirectOffsetOnAxis(ap=qidx_sb[:, t:t + 1], axis=0),
                            ctx_all[:, t * O:(t + 1) * O], None, nextq(),
                            shape_override=(T, O),
                        )

    nc.compile()
    return nc


def _get_runner():
    """Build (once) a reusable jitted SPMD callable over 8 cores."""
    with _lock:
        if "runner" in _state:
            return _state["runner"]

        import jax
        from jax.sharding import Mesh, PartitionSpec
        from jax.experimental.shard_map import shard_map
        from concourse import mybir
        from concourse import bass2jax

        nc = _build()
        bass2jax.install_neuronx_cc_hook()

        partition_name = (
            nc.partition_id_tensor.name if nc.partition_id_tensor else None
        )
        in_names, out_names, out_avals, zero_outs = [], [], [], []
        for alloc in nc.m.functions[0].allocations:
            if not isinstance(alloc, mybir.MemoryLocationSet):
                continue
            name = alloc.memorylocations[0].name
            if alloc.kind == "ExternalInput":
                if name != partition_name:
                    in_names.append(name)
            elif alloc.kind == "ExternalOutput":
                out_names.append(name)
                shape = tuple(alloc.tensor_shape)
                dtype = mybir.dt.np(alloc.dtype)
                out_avals.append(jax.core.ShapedArray(shape, dtype))
                zero_outs.append(np.zeros(shape, dtype))
        n_params = len(in_names)
        all_names = in_names + out_names
        if partition_name is not None:
            all_names = all_names + [partition_name]

        def _body(*args):
            operands = list(args)
            if partition_name is not None:
                operands.append(bass2jax.partition_id_tensor())
            outs = bass2jax._bass_exec_p.bind(
                *operands,
                out_avals=tuple(out_avals),
                in_names=tuple(all_names),
                out_names=tuple(out_names),
                lowering_input_output_aliases=(),
                sim_require_finite=True,
                sim_require_nnan=True,
                nc=nc,
            )
            return tuple(outs)

        try:
            devices = jax.devices("axon")[:N_CORES]
        except RuntimeError:
            devices = jax.devices()[:N_CORES]
        mesh = Mesh(np.asarray(devices), ("core",))
        n_out = len(out_names)
        sharded = jax.jit(
            shard_map(
                _body, mesh=mesh,
                in_specs=(PartitionSpec("core"),) * (n_params + n_out),
                out_specs=(PartitionSpec("core"),) * n_out,
                check_rep=False,
            ),
            donate_argnums=tuple(range(n_params, n_params + n_out)),
            keep_unused=True,
        )

        def run(in_maps):
            concat_in = [
                np.concatenate([np.asarray(in_maps[c][nm]) for c in range(N_CORES)],
                               axis=0)
                for nm in in_names
            ]
            concat_zero = [
                np.concatenate([z for _ in range(N_CORES)], axis=0) for z in zero_outs
            ]
            out_arrs = sharded(*concat_in, *concat_zero)
            out_arrs = [np.asarray(a) for a in out_arrs]
            results = []
            for c in range(N_CORES):
                m = {}
                for i, nm in enumerate(out_names):
                    sh0 = out_avals[i].shape[0]
                    m[nm] = out_arrs[i][c * sh0:(c + 1) * sh0]
                results.append(m)
            return results

        _state["runner"] = run
        return run


def _shard_inputs(hidden_states, attention_mask, Wq, bq, Wk, bk, Wv, bv,
                  q_indices, kv_indices):
    in_maps = []
    all_tok = np.arange(T, dtype=np.int32)
    for c in range(N_CORES):
        b, half = c // 2, c % 2
        o0 = half * O
        qi = np.ascontiguousarray(q_indices[b].astype(np.int32))
        kvi = np.ascontiguousarray(kv_indices[b].astype(np.int32))
        nqi = np.setdiff1d(all_tok, qi).astype(np.int32)
        in_maps.append({
            "hidden": np.ascontiguousarray(hidden_states[b], dtype=np.float32),
            "wqt": np.ascontiguousarray(Wq[o0:o0 + O, :].T, dtype=np.float32),
            "wkt": np.ascontiguousarray(Wk[o0:o0 + O, :].T, dtype=np.float32),
            "wvt": np.ascontiguousarray(Wv[o0:o0 + O, :].T, dtype=np.float32),
            "bq": np.ascontiguousarray(bq[o0:o0 + O], dtype=np.float32),
            "bk": np.ascontiguousarray(bk[o0:o0 + O], dtype=np.float32),
            "bv": np.ascontiguousarray(bv[o0:o0 + O], dtype=np.float32),
            "qidx": qi,
            "kvidx": kvi,
            "nqidx": nqi,
            "maskkv": np.ascontiguousarray(
                np.asarray(attention_mask, dtype=np.float32)[b, 0, 0, kvi]),
        })
    return in_maps


def kernel(hidden_states, attention_mask, Wq, bq, Wk, bk, Wv, bv,
           q_indices, kv_indices):
    run = _get_runner()
    in_maps = _shard_inputs(hidden_states, attention_mask, Wq, bq, Wk, bk, Wv, bv,
                            q_indices, kv_indices)
    results = run(in_maps)
    out = np.empty((B, T, NH * DH), dtype=np.float32)
    for c in range(N_CORES):
        b, half = c // 2, c % 2
        out[b, :, half * O:(half + 1) * O] = results[c]["out"]
    return out
